# revision 11
# baseline (speedup 1.0000x reference)
"""MiniMax Lightning Attention on 8 Trainium2 NeuronCores — bf16 edition.

Sequence-parallel: core c handles batch c//4, token chunk (c%4)*1024..+1024.
Per-chunk decay-weighted KV summaries are AllGathered (bf16); each core
reconstructs its chunk-start state as a decay-weighted sum.

All matmuls run in bf16 (fp32 PSUM accumulation). Heads are processed in
pairs packed onto partition halves (head 2c on partitions 0-63, head 2c+1
on 64-127); K=64 / M=64 attention matmuls for the two heads execute
concurrently on disjoint PE row/col groups via tile_position auto-derive.
q/k/attn stay SBUF-resident; gate activations round-trip DRAM in bf16.
Elementwise work is spread over DVE (psum-reading mask mults, state sums),
GpSimd (decayed states, gate apply), and ACT (evacuations, squares), with
one-block/one-pair lookahead so the PE never queues behind them.
"""

import numpy as np
import ml_dtypes

from contextlib import ExitStack

import concourse.bacc as bacc
import concourse.mybir as mybir
import concourse.tile as tile
from concourse.bass_utils import run_bass_kernel_spmd
from concourse.masks import make_identity

AF = mybir.ActivationFunctionType
ALU = mybir.AluOpType
F32 = mybir.dt.float32
BF = mybir.dt.bfloat16
BF_NP = ml_dtypes.bfloat16

H = 32
D = 64
BS = 256
HID = 2048
B = 2
S = 4096
NC = 8
T = S // 4            # tokens per core (1024)
NCH = T // 128        # 8 token chunks of 128
NBLK = T // BS        # 4 blocks of 256 per core
NP = H // 2           # 16 head pairs
LAYER_IDX = 0
NUM_LAYERS = 32
EPS = 1e-5


def _decay():
    base = 1.0 / 2.0 ** (8.0 / H)
    rate = base ** (np.arange(H, dtype=np.float64) + 1.0)
    factor = 1.0 - LAYER_IDX / (NUM_LAYERS - 1 + 1e-5) + 1e-5
    slope = rate * factor                                  # (H,)
    r = np.arange(BS, dtype=np.float64) + 1.0
    qd = np.exp(-slope[:, None] * r[None, :])              # (H, BS) query decay
    kd = np.exp(-slope[:, None] * (BS - r[None, :]))       # (H, BS) key decay
    ij = r[:, None] - r[None, :]                           # i - j
    dd = np.where(
        ij[None] >= 0, np.exp(-slope[:, None, None] * ij[None]), 0.0
    )                                                      # (H, BS_i, BS_j)
    bd = np.exp(-slope * BS)                               # (H,) block decay
    return slope, qd, kd, dd, bd


def _build_nc():
    nc = bacc.Bacc(num_devices=NC)
    hsT = nc.declare_dram_parameter("hsT", [HID, T], BF, isOutput=False)
    wqT = nc.declare_dram_parameter("wqT", [HID, H * D], BF, isOutput=False)
    wkT = nc.declare_dram_parameter("wkT", [HID, H * D], BF, isOutput=False)
    wvT = nc.declare_dram_parameter("wvT", [HID, H * D], BF, isOutput=False)
    gwT = nc.declare_dram_parameter("gwT", [HID, HID], BF, isOutput=False)
    owT = nc.declare_dram_parameter("owT", [H * D, HID], BF, isOutput=False)
    ddm = nc.declare_dram_parameter("ddm", [NP, 128, 2, 2, BS], BF, isOutput=False)
    qdm = nc.declare_dram_parameter("qdm", [128, NP, BS], BF, isOutput=False)
    kdm = nc.declare_dram_parameter("kdm", [128, 2 * H], F32, isOutput=False)
    swm = nc.declare_dram_parameter("swm", [128, NP * 8], F32, isOutput=False)
    bdm = nc.declare_dram_parameter("bdm", [128, NP, NBLK], F32, isOutput=False)
    out = nc.declare_dram_parameter("out", [T, HID], F32, isOutput=True)

    gate_spill = nc.dram_tensor("gate_spill", [NP, 128, T], BF)
    eloc = nc.dram_tensor("eloc", [NP, 128, D], BF)
    egath = nc.dram_tensor("egath", [NC, NP, 128, D], BF, addr_space="Shared")
    ssq_rt = nc.dram_tensor("ssq_rt", [T], F32)

    with tile.TileContext(nc, pool_alloc_mode="stack") as tc:
        # ---- constants ---------------------------------------------------
        ident_b, free_ident = tc.tile([128, 128], BF, name="ident_b")
        make_identity(nc, ident_b[:])
        ones_f, free_ones_f = tc.tile([128, 1], F32, name="ones_f")
        nc.vector.memset(ones_f[:], 1.0)
        ones_b, free_ones_b = tc.tile([128, 1], BF, name="ones_b")
        nc.scalar.copy(ones_b[:], ones_f[:])
        eps_sb, free_eps = tc.tile([128, 1], F32, name="eps_sb")
        nc.vector.memset(eps_sb[:], EPS)
        kdm_sb, free_kdm = tc.tile([128, 2 * H], F32, name="kdm_sb")
        nc.sync.dma_start(kdm_sb[:], kdm[:])
        swm_sb, free_swm = tc.tile([128, NP * 8], F32, name="swm_sb")
        nc.sync.dma_start(swm_sb[:], swm[:])
        bdm_sb, free_bdm = tc.tile([128, NP, NBLK], F32, name="bdm_sb")
        nc.sync.dma_start(bdm_sb[:], bdm[:])

        # ---- long-lived residents (stack order matters: LIFO close) ------
        _v_ctx = ExitStack()
        v_pool = _v_ctx.enter_context(tc.tile_pool(name="v_pool", bufs=1))
        V_sb = v_pool.tile([128, NCH, H * D], BF, name="V_sb")

        _pf_ctx = ExitStack()
        pf_pool = _pf_ctx.enter_context(tc.tile_pool(name="pf_pool", bufs=1))
        prefix_sb = pf_pool.tile([128, NP, NBLK - 1, D], BF, name="prefix_sb")

        _qk_ctx = ExitStack()
        qk_pool = _qk_ctx.enter_context(tc.tile_pool(name="qk_pool", bufs=1))
        qT_sb = qk_pool.tile([128, NP, T], BF, name="qT_sb")
        kT_sb = qk_pool.tile([128, NP, T], BF, name="kT_sb")

        _aq_ctx = ExitStack()
        aq_pool = _aq_ctx.enter_context(tc.tile_pool(name="aq_pool", bufs=1))
        qdq_all = aq_pool.tile([128, NP, NBLK, BS], BF, name="qdq_all")

        _xt_ctx = ExitStack()
        xt_pool = _xt_ctx.enter_context(tc.tile_pool(name="xt_pool", bufs=1))
        xT = xt_pool.tile([128, 16, T], BF, name="xT")
        for lo, hi in ((0, 1), (1, 4), (4, 8), (8, 12), (12, 16)):
            nc.sync.dma_start(
                xT[:, lo:hi, :],
                hsT[lo * 128 : hi * 128, :].rearrange("(ko p) t -> p ko t", p=128),
            )

        # ---- phase V: value projection (tok-major, all heads) ------------
        with tc.tile_pool(name="wv_p", bufs=3) as wv_p, tc.tile_pool(
            name="ps_v", bufs=1, space="PSUM"
        ) as ps_v:
            for n in range(4):
                pv = [
                    ps_v.tile([128, 512], F32, name=f"pv{m}") for m in range(NCH)
                ]
                for k in range(16):
                    wv_t = wv_p.tile([128, 512], BF, name="wv_t")
                    nc.sync.dma_start(
                        wv_t[:], wvT[k * 128 : (k + 1) * 128, n * 512 : (n + 1) * 512]
                    )
                    for m in range(NCH):
                        nc.tensor.matmul(
                            pv[m][:],
                            xT[:, k, m * 128 : (m + 1) * 128],
                            wv_t[:],
                            start=(k == 0),
                            stop=(k == 15),
                        )
                for m in range(NCH):
                    nc.scalar.activation(
                        V_sb[:, m, n * 512 : (n + 1) * 512], pv[m][:], AF.Silu
                    )

        # ---- phase QK: q/k projection per head pair + chunk summaries ----
        # one-pair lookahead: transposes/contributions of pair c-1 are
        # emitted after pair c's projection matmuls so the PE never waits
        # on the silu/copy chain.
        with tc.tile_pool(name="wq_p", bufs=2) as wq_p, tc.tile_pool(
            name="wk_p", bufs=2
        ) as wk_p, tc.tile_pool(name="kt_p", bufs=2) as kt_p, tc.tile_pool(
            name="vk_p", bufs=2
        ) as vk_p, tc.tile_pool(
            name="ef_p", bufs=2
        ) as ef_p, tc.tile_pool(
            name="ps_qk", bufs=2, space="PSUM"
        ) as ps_qk, tc.tile_pool(
            name="ps_t", bufs=1, space="PSUM"
        ) as ps_t, tc.tile_pool(
            name="ps_c", bufs=2, space="PSUM"
        ) as ps_c:
            for c in range(NP + 1):
                if c < NP:
                    wq_h = [wq_p.tile([128, 8, 128], BF, name="wq_h")
                            for _ in range(2)]
                    wk_h = [wk_p.tile([128, 8, 128], BF, name="wk_h")
                            for _ in range(2)]
                    for h in range(2):
                        nc.sync.dma_start(
                            wq_h[h][:],
                            wqT[h * 1024 : (h + 1) * 1024,
                                c * 128 : (c + 1) * 128].rearrange(
                                "(ko p) m -> p ko m", p=128
                            ),
                        )
                        nc.sync.dma_start(
                            wk_h[h][:],
                            wkT[h * 1024 : (h + 1) * 1024,
                                c * 128 : (c + 1) * 128].rearrange(
                                "(ko p) m -> p ko m", p=128
                            ),
                        )
                    for n in range(2):
                        pq = ps_qk.tile([128, 512], F32, name="pq")
                        for k in range(16):
                            nc.tensor.matmul(
                                pq[:],
                                wq_h[k // 8][:, k % 8, :],
                                xT[:, k, n * 512 : (n + 1) * 512],
                                start=(k == 0),
                                stop=(k == 15),
                            )
                        nc.scalar.activation(
                            qT_sb[:, c, n * 512 : (n + 1) * 512], pq[:], AF.Silu
                        )
                        pk = ps_qk.tile([128, 512], F32, name="pk")
                        for k in range(16):
                            nc.tensor.matmul(
                                pk[:],
                                wk_h[k // 8][:, k % 8, :],
                                xT[:, k, n * 512 : (n + 1) * 512],
                                start=(k == 0),
                                stop=(k == 15),
                            )
                        nc.scalar.activation(
                            kT_sb[:, c, n * 512 : (n + 1) * 512], pk[:], AF.Silu
                        )
                if c == 0:
                    continue
                cp = c - 1
                hA, hB = 2 * cp, 2 * cp + 1
                # k back to tok-major via PE transpose (paired row groups)
                pstA = ps_t.tile([128, 512], BF, name="pstA",
                                 padded_shape=[128, 1024])
                pstB = ps_t.tile([128, 512], BF, name="pstB",
                                 padded_shape=[128, 1024])
                for m in range(NCH):
                    nc.tensor.transpose(
                        pstA[:, m * 64 : (m + 1) * 64],
                        kT_sb[0:64, cp, m * 128 : (m + 1) * 128],
                        ident_b[0:64, 0:64],
                    )
                    nc.tensor.transpose(
                        pstB[:, m * 64 : (m + 1) * 64],
                        kT_sb[64:128, cp, m * 128 : (m + 1) * 128],
                        ident_b[64:128, 64:128],
                    )
                k_tokA = kt_p.tile([128, NCH, D], BF, name="k_tokA")
                k_tokB = kt_p.tile([128, NCH, D], BF, name="k_tokB")
                nc.scalar.copy(k_tokA[:].rearrange("p m d -> p (m d)"), pstA[:])
                nc.scalar.copy(k_tokB[:].rearrange("p m d -> p (m d)"), pstB[:])
                # v scaled by key-decay
                v_kdA = vk_p.tile([128, NCH, D], BF, name="v_kdA")
                v_kdB = vk_p.tile([128, NCH, D], BF, name="v_kdB")
                for m in range(NCH):
                    nc.vector.tensor_scalar_mul(
                        v_kdA[:, m, :],
                        V_sb[:, m, hA * D : (hA + 1) * D],
                        kdm_sb[:, 2 * hA + (m % 2) : 2 * hA + (m % 2) + 1],
                    )
                    nc.vector.tensor_scalar_mul(
                        v_kdB[:, m, :],
                        V_sb[:, m, hB * D : (hB + 1) * D],
                        kdm_sb[:, 2 * hB + (m % 2) : 2 * hB + (m % 2) + 1],
                    )
                # block contributions C_jb (paired col groups)
                pc = ps_c.tile([128, NBLK, D], F32, name="pc",
                               padded_shape=[128, NBLK, 128])
                for jb in range(NBLK):
                    for half in range(2):
                        m = 2 * jb + half
                        nc.tensor.matmul(
                            pc[0:64, jb, :],
                            k_tokA[:, m, :],
                            v_kdA[:, m, :],
                            start=(half == 0),
                            stop=(half == 1),
                        )
                        nc.tensor.matmul(
                            pc[64:128, jb, :],
                            k_tokB[:, m, :],
                            v_kdB[:, m, :],
                            start=(half == 0),
                            stop=(half == 1),
                        )
                # decay-prefix chain (both heads at once; f32 accum)
                e_f = ef_p.tile([128, D], F32, name="e_f")
                nc.vector.tensor_copy(e_f[:], pc[:, 0, :])
                nc.scalar.copy(prefix_sb[:, cp, 0, :], e_f[:])
                for jb in range(1, NBLK):
                    nc.vector.scalar_tensor_tensor(
                        e_f[:], e_f[:], bdm_sb[:, cp, 1:2], pc[:, jb, :],
                        ALU.mult, ALU.add,
                    )
                    if jb < NBLK - 1:
                        nc.scalar.copy(prefix_sb[:, cp, jb, :], e_f[:])
                e_b = ef_p.tile([128, D], BF, name="e_b")
                nc.scalar.copy(e_b[:], e_f[:])
                nc.sync.dma_start(eloc[cp], e_b[:])

        # ---- collective: share per-chunk KV summaries (bf16) -------------
        nc.gpsimd.collective_compute(
            "AllGather",
            ALU.bypass,
            replica_groups=[list(range(NC))],
            ins=[eloc[:]],
            outs=[egath[:]],
        )

        # ---- gate projection (overlaps the collective) -------------------
        # also: batched decayed-queries build + chunk-start state sums on
        # the otherwise-idle DVE, as egath arrives.
        with tc.tile_pool(name="gw_p", bufs=2) as gw_p, tc.tile_pool(
            name="gg_p", bufs=2
        ) as gg_p, tc.tile_pool(name="qdm_p", bufs=1) as qdm_p, tc.tile_pool(
            name="ps_g", bufs=2, space="PSUM"
        ) as ps_g:
            qdm_sb = qdm_p.tile([128, NP, BS], BF, name="qdm_sb")
            nc.sync.dma_start(qdm_sb[:], qdm[:])
            for gm in range(16):
                gw_t = gw_p.tile([128, 16, 128], BF, name="gw_t")
                nc.sync.dma_start(
                    gw_t[:],
                    gwT[:, gm * 128 : (gm + 1) * 128].rearrange(
                        "(ko p) g -> p ko g", p=128
                    ),
                )
                for gn in range(2):
                    pg = ps_g.tile([128, 512], F32, name="pg")
                    for gk in range(16):
                        nc.tensor.matmul(
                            pg[:],
                            gw_t[:, gk, :],
                            xT[:, gk, gn * 512 : (gn + 1) * 512],
                            start=(gk == 0),
                            stop=(gk == 15),
                        )
                    gg_t = gg_p.tile([128, 512], BF, name="gg_t")
                    nc.scalar.activation(gg_t[:], pg[:], AF.Sigmoid)
                    nc.sync.dma_start(
                        gate_spill[gm, :, gn * 512 : (gn + 1) * 512], gg_t[:]
                    )
                if gm == 0:
                    # decayed queries, all pairs per block (DVE, big ops)
                    for jb in range(NBLK):
                        nc.vector.tensor_mul(
                            qdq_all[:, :, jb, :],
                            qT_sb[:, :, jb * BS : (jb + 1) * BS],
                            qdm_sb[:],
                        )
        _xt_ctx.close()
        _ys_ctx = ExitStack()
        ys_pool = _ys_ctx.enter_context(tc.tile_pool(name="ys_pool", bufs=1))
        ys_sb = ys_pool.tile([128, 16, T], BF, name="ys_sb")

        # ---- phase A: attention per head pair ----------------------------
        # engine split per pair: PE aw/inter/intra/ssq, DVE mask mults,
        # GpSimd decayed states + gate apply, ACT evacuations + squares.
        # aw matmuls run one block ahead; sq/ssq/gate-apply run one pair
        # behind, so no engine queues behind another.
        with tc.tile_pool(name="dd_p", bufs=2) as dd_p, tc.tile_pool(
            name="gc_p", bufs=2
        ) as gc_p, tc.tile_pool(name="aw_p", bufs=2) as aw_p, tc.tile_pool(
            name="ss_p", bufs=2
        ) as ss_p, tc.tile_pool(name="sq_p", bufs=2) as sq_p, tc.tile_pool(
            name="eg_p", bufs=1
        ) as eg_p, tc.tile_pool(name="ps_aw", bufs=2, space="PSUM"
        ) as ps_aw, tc.tile_pool(
            name="ps_ys", bufs=2, space="PSUM"
        ) as ps_ys, tc.tile_pool(
            name="ps_sq", bufs=1, space="PSUM"
        ) as ps_sq:
            ssq0 = ps_sq.tile([1, 512], F32, name="ssq0")
            ssq1 = ps_sq.tile([1, 512], F32, name="ssq1")
            eg_all = eg_p.tile([128, NC, NP, D], BF, name="eg_all")
            nc.sync.dma_start(
                eg_all[:], egath[:].rearrange("cc c p e -> p cc c e")
            )
            sg0_all = eg_p.tile([128, NP, D], BF, name="sg0_all")
            gate_tiles = {}
            for c in range(NP + 1):
                if c < NP:
                    hA, hB = 2 * c, 2 * c + 1
                    dd_t = dd_p.tile([128, 2, 2, BS], BF, name="dd_t")
                    nc.sync.dma_start(dd_t[:], ddm[c])
                    gate_c = gc_p.tile([128, T], BF, name="gate_c")
                    nc.sync.dma_start(gate_c[:], gate_spill[c])
                    gate_tiles[c] = gate_c
                    # chunk-start state sum (DVE, per-pair scalar weights)
                    nc.vector.tensor_scalar_mul(
                        sg0_all[:, c, :],
                        eg_all[:, 0, c, :],
                        swm_sb[:, c * 8 : c * 8 + 1],
                    )
                    for cc in range(1, NC):
                        nc.vector.scalar_tensor_tensor(
                            sg0_all[:, c, :],
                            eg_all[:, cc, c, :],
                            swm_sb[:, c * 8 + cc : c * 8 + cc + 1],
                            sg0_all[:, c, :],
                            ALU.mult,
                            ALU.add,
                        )
                    # per-block start states (DVE; bf16 out)
                    ss_bf = ss_p.tile([128, NBLK, D], BF, name="ss_bf")
                    nc.vector.tensor_copy(ss_bf[:, 0, :], sg0_all[:, c, :])
                    for jb in range(1, NBLK):
                        nc.vector.scalar_tensor_tensor(
                            ss_bf[:, jb, :],
                            sg0_all[:, c, :],
                            bdm_sb[:, c, jb : jb + 1],
                            prefix_sb[:, c, jb - 1, :],
                            ALU.mult,
                            ALU.add,
                        )

                    def aw_block(jb, c=c):
                        pawA = ps_aw.tile([128, 2, BS], F32, name="pawA")
                        pawB = ps_aw.tile([128, 2, BS], F32, name="pawB")
                        t0 = jb * BS
                        nc.tensor.matmul(
                            pawA[:, 0, :],
                            kT_sb[0:64, c, t0 : t0 + 128],
                            qT_sb[0:64, c, t0 : t0 + BS],
                            start=True, stop=True,
                        )
                        nc.tensor.matmul(
                            pawB[:, 0, :],
                            kT_sb[64:128, c, t0 : t0 + 128],
                            qT_sb[64:128, c, t0 : t0 + BS],
                            start=True, stop=True,
                        )
                        nc.tensor.matmul(
                            pawA[:, 1, 128:256],
                            kT_sb[0:64, c, t0 + 128 : t0 + 256],
                            qT_sb[0:64, c, t0 + 128 : t0 + 256],
                            start=True, stop=True,
                        )
                        nc.tensor.matmul(
                            pawB[:, 1, 128:256],
                            kT_sb[64:128, c, t0 + 128 : t0 + 256],
                            qT_sb[64:128, c, t0 + 128 : t0 + 256],
                            start=True, stop=True,
                        )
                        return pawA, pawB

                    paws = {0: aw_block(0), 1: aw_block(1)}
                if c > 0:
                    # deferred tail of pair c-1: rmsnorm squares + gate
                    cp = c - 1
                    sq_t = sq_p.tile([128, T], BF, name="sq_t")
                    nc.scalar.activation(sq_t[:], ys_sb[:, cp, :], AF.Square)
                    nc.tensor.matmul(
                        ssq0[:], ones_b[:], sq_t[:, 0:512],
                        start=(cp == 0), stop=(cp == NP - 1),
                        skip_group_check=True,
                    )
                    nc.tensor.matmul(
                        ssq1[:], ones_b[:], sq_t[:, 512:1024],
                        start=(cp == 0), stop=(cp == NP - 1),
                        skip_group_check=True,
                    )
                    nc.gpsimd.tensor_mul(
                        ys_sb[:, cp, :], ys_sb[:, cp, :], gate_tiles.pop(cp)[:]
                    )
                if c == NP:
                    continue
                for jb in range(NBLK):
                    pawA, pawB = paws.pop(jb)
                    t0 = jb * BS
                    awmA = aw_p.tile([128, 2, BS], BF, name="awmA")
                    awmB = aw_p.tile([128, 2, BS], BF, name="awmB")
                    nc.vector.tensor_mul(
                        awmA[:, 0, :], pawA[:, 0, :], dd_t[:, 0, 0, :]
                    )
                    nc.vector.tensor_mul(
                        awmA[:, 1, 128:256], pawA[:, 1, 128:256],
                        dd_t[:, 0, 1, 128:256],
                    )
                    nc.vector.tensor_mul(
                        awmB[:, 0, :], pawB[:, 0, :], dd_t[:, 1, 0, :]
                    )
                    nc.vector.tensor_mul(
                        awmB[:, 1, 128:256], pawB[:, 1, 128:256],
                        dd_t[:, 1, 1, 128:256],
                    )
                    pys = ps_ys.tile([128, BS], F32, name="pys",
                                     padded_shape=[128, 512])
                    nc.tensor.matmul(
                        pys[0:64, :], ss_bf[0:64, jb, :], qdq_all[0:64, c, jb, :],
                        start=True, stop=False,
                    )
                    nc.tensor.matmul(
                        pys[64:128, :], ss_bf[64:128, jb, :],
                        qdq_all[64:128, c, jb, :],
                        start=True, stop=False,
                    )
                    nc.tensor.matmul(
                        pys[0:64, :],
                        V_sb[:, 2 * jb, hA * D : (hA + 1) * D],
                        awmA[:, 0, :],
                        start=False, stop=False,
                    )
                    nc.tensor.matmul(
                        pys[64:128, :],
                        V_sb[:, 2 * jb, hB * D : (hB + 1) * D],
                        awmB[:, 0, :],
                        start=False, stop=False,
                    )
                    nc.tensor.matmul(
                        pys[0:64, 128:256],
                        V_sb[:, 2 * jb + 1, hA * D : (hA + 1) * D],
                        awmA[:, 1, 128:256],
                        start=False, stop=True,
                    )
                    nc.tensor.matmul(
                        pys[64:128, 128:256],
                        V_sb[:, 2 * jb + 1, hB * D : (hB + 1) * D],
                        awmB[:, 1, 128:256],
                        start=False, stop=True,
                    )
                    if jb + 2 < NBLK:
                        paws[jb + 2] = aw_block(jb + 2)
                    nc.scalar.copy(ys_sb[:, c, t0 : t0 + BS], pys[:])
            # ssq round-trip for per-token layout
            with tc.tile_pool(name="ns_p", bufs=1) as ns_p:
                ssq_sb = ns_p.tile([1, T], F32, name="ssq_sb")
                nc.vector.tensor_copy(ssq_sb[:, 0:512], ssq0[:])
                nc.vector.tensor_copy(ssq_sb[:, 512:1024], ssq1[:])
                nc.sync.dma_start(ssq_rt[:], ssq_sb[:])

        # ---- phase F: output projection ----------------------------------
        with tc.tile_pool(name="nsv_p", bufs=1) as nsv_p:
            ns_l = nsv_p.tile([128, NCH], F32, name="ns_l")
            nc.sync.dma_start(ns_l[:], ssq_rt.rearrange("(c p) -> p c", p=128))
            ns_t = nsv_p.tile([128, NCH], F32, name="ns_t")
            nc.scalar.activation(
                ns_t[:], ns_l[:], AF.Sqrt, bias=eps_sb[:, 0:1], scale=1.0 / (H * D)
            )
            ns_sb = nsv_p.tile([128, NCH], F32, name="ns_sb")
            nc.vector.reciprocal(ns_sb[:], ns_t[:])

            with tc.tile_pool(name="ow_p", bufs=2) as ow_p, tc.tile_pool(
                name="oo_p", bufs=3
            ) as oo_p, tc.tile_pool(name="ps_o", bufs=3, space="PSUM") as ps_o:
                for n in range(4):
                    ow_h = [ow_p.tile([128, 8, 512], BF, name="ow_h")
                            for _ in range(2)]
                    for h in range(2):
                        nc.sync.dma_start(
                            ow_h[h][:],
                            owT[h * 1024 : (h + 1) * 1024,
                                n * 512 : (n + 1) * 512].rearrange(
                                "(ko p) f -> p ko f", p=128
                            ),
                        )
                    for m in range(NCH):
                        po = ps_o.tile([128, 512], F32, name="po")
                        for k in range(16):
                            nc.tensor.matmul(
                                po[:],
                                ys_sb[:, k, m * 128 : (m + 1) * 128],
                                ow_h[k // 8][:, k % 8, :],
                                start=(k == 0),
                                stop=(k == 15),
                            )
                        oo_t = oo_p.tile([128, 512], F32, name="oo_t")
                        nc.scalar.mul(oo_t[:], po[:], ns_sb[:, m : m + 1])
                        nc.sync.dma_start(
                            out[m * 128 : (m + 1) * 128, n * 512 : (n + 1) * 512],
                            oo_t[:],
                        )
        _ys_ctx.close()
        _aq_ctx.close()
        _qk_ctx.close()
        _pf_ctx.close()
        _v_ctx.close()
        free_bdm()
        free_swm()
        free_kdm()
        free_eps()
        free_ones_b()
        free_ones_f()
        free_ident()
    nc.finalize()
    return nc


_CACHE = {}


def _get_nc():
    if "nc" not in _CACHE:
        _CACHE["nc"] = _build_nc()
    return _CACHE["nc"]


def _host_prep(hidden_states, qkv_w, out_w, gate_w, norm_w):
    slope, qd, kd, dd, bd = _decay()
    w3 = qkv_w.reshape(H, 3 * D, HID)
    wq = w3[:, 0:D, :].reshape(H * D, HID)
    wk = w3[:, D : 2 * D, :].reshape(H * D, HID)
    wv = w3[:, 2 * D : 3 * D, :].reshape(H * D, HID)
    wqT = np.ascontiguousarray(wq.T.astype(BF_NP))
    wkT = np.ascontiguousarray(wk.T.astype(BF_NP))
    wvT = np.ascontiguousarray(wv.T.astype(BF_NP))
    gwT = np.ascontiguousarray(gate_w.T.astype(BF_NP))
    # norm_w folded into the output projection (rmsnorm's per-token scale
    # is applied at PSUM evacuation; the per-feature nw scales ow columns)
    owT = np.ascontiguousarray((out_w * norm_w[None, :]).T.astype(BF_NP))
    # ddm[c, p, e, jc, i] = dd[2c+e, i, jc*128+p]
    ddm = np.ascontiguousarray(
        dd.reshape(NP, 2, BS, 2, 128).transpose(0, 4, 1, 3, 2).astype(BF_NP)
    )
    # qdm[p, c, i] = qd[2c + (p>=64), i]
    qdm_pair = qd.reshape(NP, 2, BS)                       # (c, e, i)
    qdm = np.ascontiguousarray(
        np.broadcast_to(
            qdm_pair.transpose(1, 0, 2)[:, None, :, :], (2, 64, NP, BS)
        ).reshape(128, NP, BS).astype(BF_NP)
    )
    kdm = np.ascontiguousarray(
        kd.reshape(H, 2, 128).transpose(2, 0, 1).reshape(128, 2 * H).astype(np.float32)
    )
    # bdm[p, c, jb] = bd[2c + (p>=64)]^jb
    jbp = np.arange(NBLK, dtype=np.float64)
    bdp = bd[:, None] ** jbp[None, :]                      # (H, NBLK)
    bdm = np.ascontiguousarray(
        np.broadcast_to(
            bdp.reshape(NP, 2, NBLK).transpose(1, 0, 2)[:, None, :, :],
            (2, 64, NP, NBLK),
        ).reshape(128, NP, NBLK).astype(np.float32)
    )

    shared = dict(wqT=wqT, wkT=wkT, wvT=wvT, gwT=gwT, owT=owT, ddm=ddm,
                  qdm=qdm, kdm=kdm, bdm=bdm)
    in_maps = []
    for c in range(NC):
        bb, p = c // 4, c % 4
        hsT = np.ascontiguousarray(
            hidden_states[bb, p * T : (p + 1) * T, :].T.astype(BF_NP)
        )
        # swm[p_, c_*8+cc] = weight of core cc for head 2c_+(p_>=64)
        sw = np.zeros((H, NC), dtype=np.float64)
        for cc in range(NC):
            if cc // 4 == bb and cc % 4 < p:
                sw[:, cc] = bd ** (4.0 * (p - 1 - (cc % 4)))
        swm = np.ascontiguousarray(
            np.broadcast_to(
                sw.reshape(NP, 2, NC).transpose(1, 0, 2)[:, None, :, :],
                (2, 64, NP, NC),
            ).reshape(128, NP * 8).astype(np.float32)
        )
        in_maps.append(dict(hsT=hsT, swm=swm, **shared))
    return in_maps


def _run(inputs, trace=False):
    nc = _get_nc()
    in_maps = _host_prep(
        np.asarray(inputs["hidden_states"], dtype=np.float32),
        np.asarray(inputs["qkv_w"], dtype=np.float32),
        np.asarray(inputs["out_w"], dtype=np.float32),
        np.asarray(inputs["gate_w"], dtype=np.float32),
        np.asarray(inputs["norm_w"], dtype=np.float32),
    )
    res = run_bass_kernel_spmd(nc, in_maps, core_ids=list(range(NC)), trace=trace)
    full = np.empty((B, S, HID), dtype=np.float32)
    for c in range(NC):
        bb, p = c // 4, c % 4
        full[bb, p * T : (p + 1) * T, :] = res.results[c]["out"]
    return full, res


def kernel(**inputs):
    return _run(inputs, trace=False)[0]


def kernel_traced(**inputs):
    full, res = _run(inputs, trace=True)
    return full, res.exec_time_ns


# revision 12
# speedup vs baseline: 1.0304x; 1.0304x over previous
"""MiniMax Lightning Attention on 8 Trainium2 NeuronCores — bf16 edition.

Sequence-parallel: core c handles batch c//4, token chunk (c%4)*1024..+1024.
Per-chunk decay-weighted KV summaries are AllGathered (bf16); each core
reconstructs its chunk-start state as a decay-weighted sum.

All matmuls run in bf16 (fp32 PSUM accumulation). Heads are processed in
pairs packed onto partition halves (head 2c on partitions 0-63, head 2c+1
on 64-127); K=64 / M=64 attention matmuls for the two heads execute
concurrently on disjoint PE row/col groups via tile_position auto-derive.
q/k/attn stay SBUF-resident; gate activations round-trip DRAM in bf16.
Elementwise work is spread over DVE (psum-reading mask mults, state sums),
GpSimd (decayed states, gate apply), and ACT (evacuations, squares), with
one-block/one-pair lookahead so the PE never queues behind them.
"""

import numpy as np
import ml_dtypes

from contextlib import ExitStack

import concourse.bacc as bacc
import concourse.mybir as mybir
import concourse.tile as tile
from concourse.bass_utils import run_bass_kernel_spmd
from concourse.masks import make_identity

AF = mybir.ActivationFunctionType
ALU = mybir.AluOpType
F32 = mybir.dt.float32
BF = mybir.dt.bfloat16
BF_NP = ml_dtypes.bfloat16

H = 32
D = 64
BS = 256
HID = 2048
B = 2
S = 4096
NC = 8
T = S // 4            # tokens per core (1024)
NCH = T // 128        # 8 token chunks of 128
NBLK = T // BS        # 4 blocks of 256 per core
NP = H // 2           # 16 head pairs
LAYER_IDX = 0
NUM_LAYERS = 32
EPS = 1e-5


def _decay():
    base = 1.0 / 2.0 ** (8.0 / H)
    rate = base ** (np.arange(H, dtype=np.float64) + 1.0)
    factor = 1.0 - LAYER_IDX / (NUM_LAYERS - 1 + 1e-5) + 1e-5
    slope = rate * factor                                  # (H,)
    r = np.arange(BS, dtype=np.float64) + 1.0
    qd = np.exp(-slope[:, None] * r[None, :])              # (H, BS) query decay
    kd = np.exp(-slope[:, None] * (BS - r[None, :]))       # (H, BS) key decay
    ij = r[:, None] - r[None, :]                           # i - j
    dd = np.where(
        ij[None] >= 0, np.exp(-slope[:, None, None] * ij[None]), 0.0
    )                                                      # (H, BS_i, BS_j)
    bd = np.exp(-slope * BS)                               # (H,) block decay
    return slope, qd, kd, dd, bd


def _build_nc():
    nc = bacc.Bacc(num_devices=NC)
    hsT = nc.declare_dram_parameter("hsT", [HID, T], BF, isOutput=False)
    wqT = nc.declare_dram_parameter("wqT", [HID, H * D], BF, isOutput=False)
    wkT = nc.declare_dram_parameter("wkT", [HID, H * D], BF, isOutput=False)
    wvT = nc.declare_dram_parameter("wvT", [HID, H * D], BF, isOutput=False)
    gwT = nc.declare_dram_parameter("gwT", [HID, HID], BF, isOutput=False)
    owT = nc.declare_dram_parameter("owT", [H * D, HID], BF, isOutput=False)
    ddm = nc.declare_dram_parameter("ddm", [NP, 128, 2, 2, BS], F32, isOutput=False)
    qdm = nc.declare_dram_parameter("qdm", [128, NP, BS], BF, isOutput=False)
    kdm = nc.declare_dram_parameter("kdm", [128, 2 * H], F32, isOutput=False)
    swm = nc.declare_dram_parameter("swm", [128, NP * 8], F32, isOutput=False)
    bdm = nc.declare_dram_parameter("bdm", [128, NP, NBLK], F32, isOutput=False)
    out = nc.declare_dram_parameter("out", [T, HID], F32, isOutput=True)

    gate_spill = nc.dram_tensor("gate_spill", [NP, 128, T], BF)
    eloc = nc.dram_tensor("eloc", [128, NP * D], F32)
    egath = nc.dram_tensor("egath", [NC, 128, NP * D], F32, addr_space="Shared")
    ssq_rt = nc.dram_tensor("ssq_rt", [T], F32)

    with tile.TileContext(nc, pool_alloc_mode="stack") as tc:
        # ---- constants ---------------------------------------------------
        ident_b, free_ident = tc.tile([128, 128], BF, name="ident_b")
        make_identity(nc, ident_b[:])
        ones_f, free_ones_f = tc.tile([128, 1], F32, name="ones_f")
        nc.vector.memset(ones_f[:], 1.0)
        ones_b, free_ones_b = tc.tile([128, 1], BF, name="ones_b")
        nc.scalar.copy(ones_b[:], ones_f[:])
        eps_sb, free_eps = tc.tile([128, 1], F32, name="eps_sb")
        nc.vector.memset(eps_sb[:], EPS)
        kdm_sb, free_kdm = tc.tile([128, 2 * H], F32, name="kdm_sb")
        nc.sync.dma_start(kdm_sb[:], kdm[:])
        swm_sb, free_swm = tc.tile([128, NP * 8], F32, name="swm_sb")
        nc.sync.dma_start(swm_sb[:], swm[:])
        bdm_sb, free_bdm = tc.tile([128, NP, NBLK], F32, name="bdm_sb")
        nc.sync.dma_start(bdm_sb[:], bdm[:])

        # ---- long-lived residents (stack order matters: LIFO close) ------
        _v_ctx = ExitStack()
        v_pool = _v_ctx.enter_context(tc.tile_pool(name="v_pool", bufs=1))
        V_sb = v_pool.tile([128, NCH, H * D], BF, name="V_sb")

        _pf_ctx = ExitStack()
        pf_pool = _pf_ctx.enter_context(tc.tile_pool(name="pf_pool", bufs=1))
        prefix_sb = pf_pool.tile([128, NP, NBLK - 1, D], F32, name="prefix_sb")

        _qk_ctx = ExitStack()
        qk_pool = _qk_ctx.enter_context(tc.tile_pool(name="qk_pool", bufs=1))
        qT_sb = qk_pool.tile([128, NP, T], BF, name="qT_sb")
        kT_sb = qk_pool.tile([128, NP, T], BF, name="kT_sb")
        e_all = qk_pool.tile([128, NP, D], F32, name="e_all")

        _aq_ctx = ExitStack()
        aq_pool = _aq_ctx.enter_context(tc.tile_pool(name="aq_pool", bufs=1))
        qdq_all = aq_pool.tile([128, NP, NBLK, BS], BF, name="qdq_all")

        _xt_ctx = ExitStack()
        xt_pool = _xt_ctx.enter_context(tc.tile_pool(name="xt_pool", bufs=1))
        xT = xt_pool.tile([128, 16, T], BF, name="xT")
        for lo, hi in ((0, 1), (1, 4), (4, 8), (8, 12), (12, 16)):
            nc.sync.dma_start(
                xT[:, lo:hi, :],
                hsT[lo * 128 : hi * 128, :].rearrange("(ko p) t -> p ko t", p=128),
            )

        # ---- phase V: value projection (tok-major, all heads) ------------
        with tc.tile_pool(name="wv_p", bufs=3) as wv_p, tc.tile_pool(
            name="ps_v", bufs=1, space="PSUM"
        ) as ps_v:
            for n in range(4):
                pv = [
                    ps_v.tile([128, 512], F32, name=f"pv{m}") for m in range(NCH)
                ]
                for k in range(16):
                    wv_t = wv_p.tile([128, 512], BF, name="wv_t")
                    nc.sync.dma_start(
                        wv_t[:], wvT[k * 128 : (k + 1) * 128, n * 512 : (n + 1) * 512]
                    )
                    for m in range(NCH):
                        nc.tensor.matmul(
                            pv[m][:],
                            xT[:, k, m * 128 : (m + 1) * 128],
                            wv_t[:],
                            start=(k == 0),
                            stop=(k == 15),
                        )
                for m in range(NCH):
                    nc.scalar.activation(
                        V_sb[:, m, n * 512 : (n + 1) * 512], pv[m][:], AF.Silu
                    )

        # ---- phase QK: q/k projection per head pair + chunk summaries ----
        # one-pair lookahead: transposes/contributions of pair c-1 are
        # emitted after pair c's projection matmuls so the PE never waits
        # on the silu/copy chain.
        with tc.tile_pool(name="wq_p", bufs=2) as wq_p, tc.tile_pool(
            name="wk_p", bufs=2
        ) as wk_p, tc.tile_pool(name="kt_p", bufs=2) as kt_p, tc.tile_pool(
            name="vk_p", bufs=2
        ) as vk_p, tc.tile_pool(
            name="ef_p", bufs=2
        ) as ef_p, tc.tile_pool(
            name="ps_qk", bufs=2, space="PSUM"
        ) as ps_qk, tc.tile_pool(
            name="ps_t", bufs=1, space="PSUM"
        ) as ps_t, tc.tile_pool(
            name="ps_c", bufs=2, space="PSUM"
        ) as ps_c:
            for c in range(NP + 1):
                if c < NP:
                    wq_h = [wq_p.tile([128, 8, 128], BF, name="wq_h")
                            for _ in range(2)]
                    wk_h = [wk_p.tile([128, 8, 128], BF, name="wk_h")
                            for _ in range(2)]
                    for h in range(2):
                        nc.sync.dma_start(
                            wq_h[h][:],
                            wqT[h * 1024 : (h + 1) * 1024,
                                c * 128 : (c + 1) * 128].rearrange(
                                "(ko p) m -> p ko m", p=128
                            ),
                        )
                        nc.sync.dma_start(
                            wk_h[h][:],
                            wkT[h * 1024 : (h + 1) * 1024,
                                c * 128 : (c + 1) * 128].rearrange(
                                "(ko p) m -> p ko m", p=128
                            ),
                        )
                    for n in range(2):
                        pq = ps_qk.tile([128, 512], F32, name="pq")
                        for k in range(16):
                            nc.tensor.matmul(
                                pq[:],
                                wq_h[k // 8][:, k % 8, :],
                                xT[:, k, n * 512 : (n + 1) * 512],
                                start=(k == 0),
                                stop=(k == 15),
                            )
                        nc.scalar.activation(
                            qT_sb[:, c, n * 512 : (n + 1) * 512], pq[:], AF.Silu
                        )
                        pk = ps_qk.tile([128, 512], F32, name="pk")
                        for k in range(16):
                            nc.tensor.matmul(
                                pk[:],
                                wk_h[k // 8][:, k % 8, :],
                                xT[:, k, n * 512 : (n + 1) * 512],
                                start=(k == 0),
                                stop=(k == 15),
                            )
                        nc.scalar.activation(
                            kT_sb[:, c, n * 512 : (n + 1) * 512], pk[:], AF.Silu
                        )
                if c == 0:
                    continue
                cp = c - 1
                hA, hB = 2 * cp, 2 * cp + 1
                # k back to tok-major via PE transpose (paired row groups)
                pstA = ps_t.tile([128, 512], BF, name="pstA",
                                 padded_shape=[128, 1024])
                pstB = ps_t.tile([128, 512], BF, name="pstB",
                                 padded_shape=[128, 1024])
                for m in range(NCH):
                    nc.tensor.transpose(
                        pstA[:, m * 64 : (m + 1) * 64],
                        kT_sb[0:64, cp, m * 128 : (m + 1) * 128],
                        ident_b[0:64, 0:64],
                    )
                    nc.tensor.transpose(
                        pstB[:, m * 64 : (m + 1) * 64],
                        kT_sb[64:128, cp, m * 128 : (m + 1) * 128],
                        ident_b[64:128, 64:128],
                    )
                k_tokA = kt_p.tile([128, NCH, D], BF, name="k_tokA")
                k_tokB = kt_p.tile([128, NCH, D], BF, name="k_tokB")
                nc.scalar.copy(k_tokA[:].rearrange("p m d -> p (m d)"), pstA[:])
                nc.scalar.copy(k_tokB[:].rearrange("p m d -> p (m d)"), pstB[:])
                # v scaled by key-decay
                v_kdA = vk_p.tile([128, NCH, D], BF, name="v_kdA")
                v_kdB = vk_p.tile([128, NCH, D], BF, name="v_kdB")
                for m in range(NCH):
                    nc.vector.tensor_scalar_mul(
                        v_kdA[:, m, :],
                        V_sb[:, m, hA * D : (hA + 1) * D],
                        kdm_sb[:, 2 * hA + (m % 2) : 2 * hA + (m % 2) + 1],
                    )
                    nc.vector.tensor_scalar_mul(
                        v_kdB[:, m, :],
                        V_sb[:, m, hB * D : (hB + 1) * D],
                        kdm_sb[:, 2 * hB + (m % 2) : 2 * hB + (m % 2) + 1],
                    )
                # block contributions C_jb (paired col groups)
                pc = ps_c.tile([128, NBLK, D], F32, name="pc",
                               padded_shape=[128, NBLK, 128])
                for jb in range(NBLK):
                    for half in range(2):
                        m = 2 * jb + half
                        nc.tensor.matmul(
                            pc[0:64, jb, :],
                            k_tokA[:, m, :],
                            v_kdA[:, m, :],
                            start=(half == 0),
                            stop=(half == 1),
                        )
                        nc.tensor.matmul(
                            pc[64:128, jb, :],
                            k_tokB[:, m, :],
                            v_kdB[:, m, :],
                            start=(half == 0),
                            stop=(half == 1),
                        )
                # decay-prefix chain (both heads at once; f32 accum)
                e_f = ef_p.tile([128, D], F32, name="e_f")
                nc.vector.tensor_copy(e_f[:], pc[:, 0, :])
                nc.scalar.copy(prefix_sb[:, cp, 0, :], e_f[:])
                for jb in range(1, NBLK):
                    nc.vector.scalar_tensor_tensor(
                        e_f[:], e_f[:], bdm_sb[:, cp, 1:2], pc[:, jb, :],
                        ALU.mult, ALU.add,
                    )
                    if jb < NBLK - 1:
                        nc.scalar.copy(prefix_sb[:, cp, jb, :], e_f[:])
                nc.scalar.copy(e_all[:, cp, :], e_f[:])
                if cp == NP - 1:
                    nc.sync.dma_start(
                        eloc[:], e_all[:].rearrange("p c e -> p (c e)")
                    )

        # ---- collective: share per-chunk KV summaries (bf16) -------------
        nc.gpsimd.collective_compute(
            "AllGather",
            ALU.bypass,
            replica_groups=[list(range(NC))],
            ins=[eloc[:]],
            outs=[egath[:]],
        )

        # ---- gate projection (overlaps the collective) -------------------
        # also: batched decayed-queries build + chunk-start state sums on
        # the otherwise-idle DVE, as egath arrives.
        with tc.tile_pool(name="gw_p", bufs=2) as gw_p, tc.tile_pool(
            name="gg_p", bufs=2
        ) as gg_p, tc.tile_pool(name="qdm_p", bufs=1) as qdm_p, tc.tile_pool(
            name="ps_g", bufs=2, space="PSUM"
        ) as ps_g:
            qdm_sb = qdm_p.tile([128, NP, BS], BF, name="qdm_sb")
            nc.sync.dma_start(qdm_sb[:], qdm[:])
            for gm in range(16):
                gw_t = gw_p.tile([128, 16, 128], BF, name="gw_t")
                nc.sync.dma_start(
                    gw_t[:],
                    gwT[:, gm * 128 : (gm + 1) * 128].rearrange(
                        "(ko p) g -> p ko g", p=128
                    ),
                )
                for gn in range(2):
                    pg = ps_g.tile([128, 512], F32, name="pg")
                    for gk in range(16):
                        nc.tensor.matmul(
                            pg[:],
                            gw_t[:, gk, :],
                            xT[:, gk, gn * 512 : (gn + 1) * 512],
                            start=(gk == 0),
                            stop=(gk == 15),
                        )
                    gg_t = gg_p.tile([128, 512], BF, name="gg_t")
                    nc.scalar.activation(gg_t[:], pg[:], AF.Sigmoid)
                    nc.sync.dma_start(
                        gate_spill[gm, :, gn * 512 : (gn + 1) * 512], gg_t[:]
                    )
                if gm == 0:
                    # decayed queries, all pairs per block (DVE, big ops)
                    for jb in range(NBLK):
                        nc.vector.tensor_mul(
                            qdq_all[:, :, jb, :],
                            qT_sb[:, :, jb * BS : (jb + 1) * BS],
                            qdm_sb[:],
                        )
        _xt_ctx.close()
        _ys_ctx = ExitStack()
        ys_pool = _ys_ctx.enter_context(tc.tile_pool(name="ys_pool", bufs=1))
        ys_sb = ys_pool.tile([128, 16, T], BF, name="ys_sb")

        # ---- phase A: attention per head pair ----------------------------
        # engine split per pair: PE aw/inter/intra/ssq, DVE mask mults,
        # GpSimd decayed states + gate apply, ACT evacuations + squares.
        # aw matmuls run one block ahead; sq/ssq/gate-apply run one pair
        # behind, so no engine queues behind another.
        with tc.tile_pool(name="dd_p", bufs=2) as dd_p, tc.tile_pool(
            name="gc_p", bufs=2
        ) as gc_p, tc.tile_pool(name="aw_p", bufs=2) as aw_p, tc.tile_pool(
            name="ss_p", bufs=2
        ) as ss_p, tc.tile_pool(name="sq_p", bufs=2) as sq_p, tc.tile_pool(
            name="eg_p", bufs=2
        ) as eg_p, tc.tile_pool(name="ps_aw", bufs=2, space="PSUM"
        ) as ps_aw, tc.tile_pool(
            name="ps_ys", bufs=2, space="PSUM"
        ) as ps_ys, tc.tile_pool(
            name="ps_sq", bufs=1, space="PSUM"
        ) as ps_sq:
            ssq0 = ps_sq.tile([1, 512], F32, name="ssq0")
            ssq1 = ps_sq.tile([1, 512], F32, name="ssq1")
            gate_tiles = {}
            for c in range(NP + 1):
                if c < NP:
                    hA, hB = 2 * c, 2 * c + 1
                    dd_t = dd_p.tile([128, 2, 2, BS], F32, name="dd_t")
                    nc.sync.dma_start(dd_t[:], ddm[c])
                    gate_c = gc_p.tile([128, T], BF, name="gate_c")
                    nc.sync.dma_start(gate_c[:], gate_spill[c])
                    gate_tiles[c] = gate_c
                    # chunk-start state sum (DVE, per-pair scalar weights)
                    eg_t = eg_p.tile([128, NC, D], F32, name="eg_t")
                    nc.sync.dma_start(
                        eg_t[:],
                        egath[:, :, c * D : (c + 1) * D].rearrange(
                            "cc p e -> p cc e"
                        ),
                    )
                    sg0 = ss_p.tile([128, D], F32, name="sg0")
                    nc.vector.tensor_scalar_mul(
                        sg0[:], eg_t[:, 0, :], swm_sb[:, c * 8 : c * 8 + 1]
                    )
                    for cc in range(1, NC):
                        nc.vector.scalar_tensor_tensor(
                            sg0[:],
                            eg_t[:, cc, :],
                            swm_sb[:, c * 8 + cc : c * 8 + cc + 1],
                            sg0[:],
                            ALU.mult,
                            ALU.add,
                        )
                    # per-block start states (DVE; bf16 out)
                    ss_bf = ss_p.tile([128, NBLK, D], BF, name="ss_bf")
                    nc.vector.tensor_copy(ss_bf[:, 0, :], sg0[:])
                    for jb in range(1, NBLK):
                        nc.vector.scalar_tensor_tensor(
                            ss_bf[:, jb, :],
                            sg0[:],
                            bdm_sb[:, c, jb : jb + 1],
                            prefix_sb[:, c, jb - 1, :],
                            ALU.mult,
                            ALU.add,
                        )

                    def aw_block(jb, c=c):
                        pawA = ps_aw.tile([128, 2, BS], F32, name="pawA")
                        pawB = ps_aw.tile([128, 2, BS], F32, name="pawB")
                        t0 = jb * BS
                        nc.tensor.matmul(
                            pawA[:, 0, :],
                            kT_sb[0:64, c, t0 : t0 + 128],
                            qT_sb[0:64, c, t0 : t0 + BS],
                            start=True, stop=True,
                        )
                        nc.tensor.matmul(
                            pawB[:, 0, :],
                            kT_sb[64:128, c, t0 : t0 + 128],
                            qT_sb[64:128, c, t0 : t0 + BS],
                            start=True, stop=True,
                        )
                        nc.tensor.matmul(
                            pawA[:, 1, 128:256],
                            kT_sb[0:64, c, t0 + 128 : t0 + 256],
                            qT_sb[0:64, c, t0 + 128 : t0 + 256],
                            start=True, stop=True,
                        )
                        nc.tensor.matmul(
                            pawB[:, 1, 128:256],
                            kT_sb[64:128, c, t0 + 128 : t0 + 256],
                            qT_sb[64:128, c, t0 + 128 : t0 + 256],
                            start=True, stop=True,
                        )
                        return pawA, pawB

                    paws = {0: aw_block(0), 1: aw_block(1)}
                if c > 0:
                    # deferred tail of pair c-1: rmsnorm squares + gate
                    cp = c - 1
                    sq_t = sq_p.tile([128, T], BF, name="sq_t")
                    nc.scalar.activation(sq_t[:], ys_sb[:, cp, :], AF.Square)
                    nc.tensor.matmul(
                        ssq0[:], ones_b[:], sq_t[:, 0:512],
                        start=(cp == 0), stop=(cp == NP - 1),
                        skip_group_check=True,
                    )
                    nc.tensor.matmul(
                        ssq1[:], ones_b[:], sq_t[:, 512:1024],
                        start=(cp == 0), stop=(cp == NP - 1),
                        skip_group_check=True,
                    )
                    nc.gpsimd.tensor_mul(
                        ys_sb[:, cp, :], ys_sb[:, cp, :], gate_tiles.pop(cp)[:]
                    )
                if c == NP:
                    continue
                for jb in range(NBLK):
                    pawA, pawB = paws.pop(jb)
                    t0 = jb * BS
                    awmA = aw_p.tile([128, 2, BS], BF, name="awmA")
                    awmB = aw_p.tile([128, 2, BS], BF, name="awmB")
                    nc.vector.tensor_mul(
                        awmA[:, 0, :], pawA[:, 0, :], dd_t[:, 0, 0, :]
                    )
                    nc.vector.tensor_mul(
                        awmA[:, 1, 128:256], pawA[:, 1, 128:256],
                        dd_t[:, 0, 1, 128:256],
                    )
                    nc.vector.tensor_mul(
                        awmB[:, 0, :], pawB[:, 0, :], dd_t[:, 1, 0, :]
                    )
                    nc.vector.tensor_mul(
                        awmB[:, 1, 128:256], pawB[:, 1, 128:256],
                        dd_t[:, 1, 1, 128:256],
                    )
                    pys = ps_ys.tile([128, BS], F32, name="pys",
                                     padded_shape=[128, 512])
                    nc.tensor.matmul(
                        pys[0:64, :], ss_bf[0:64, jb, :], qdq_all[0:64, c, jb, :],
                        start=True, stop=False,
                    )
                    nc.tensor.matmul(
                        pys[64:128, :], ss_bf[64:128, jb, :],
                        qdq_all[64:128, c, jb, :],
                        start=True, stop=False,
                    )
                    nc.tensor.matmul(
                        pys[0:64, :],
                        V_sb[:, 2 * jb, hA * D : (hA + 1) * D],
                        awmA[:, 0, :],
                        start=False, stop=False,
                    )
                    nc.tensor.matmul(
                        pys[64:128, :],
                        V_sb[:, 2 * jb, hB * D : (hB + 1) * D],
                        awmB[:, 0, :],
                        start=False, stop=False,
                    )
                    nc.tensor.matmul(
                        pys[0:64, 128:256],
                        V_sb[:, 2 * jb + 1, hA * D : (hA + 1) * D],
                        awmA[:, 1, 128:256],
                        start=False, stop=True,
                    )
                    nc.tensor.matmul(
                        pys[64:128, 128:256],
                        V_sb[:, 2 * jb + 1, hB * D : (hB + 1) * D],
                        awmB[:, 1, 128:256],
                        start=False, stop=True,
                    )
                    if jb + 2 < NBLK:
                        paws[jb + 2] = aw_block(jb + 2)
                    nc.scalar.copy(ys_sb[:, c, t0 : t0 + BS], pys[:])
            # ssq round-trip for per-token layout
            with tc.tile_pool(name="ns_p", bufs=1) as ns_p:
                ssq_sb = ns_p.tile([1, T], F32, name="ssq_sb")
                nc.vector.tensor_copy(ssq_sb[:, 0:512], ssq0[:])
                nc.vector.tensor_copy(ssq_sb[:, 512:1024], ssq1[:])
                nc.sync.dma_start(ssq_rt[:], ssq_sb[:])

        # ---- phase F: output projection ----------------------------------
        with tc.tile_pool(name="nsv_p", bufs=1) as nsv_p:
            ns_l = nsv_p.tile([128, NCH], F32, name="ns_l")
            nc.sync.dma_start(ns_l[:], ssq_rt.rearrange("(c p) -> p c", p=128))
            ns_t = nsv_p.tile([128, NCH], F32, name="ns_t")
            nc.scalar.activation(
                ns_t[:], ns_l[:], AF.Sqrt, bias=eps_sb[:, 0:1], scale=1.0 / (H * D)
            )
            ns_sb = nsv_p.tile([128, NCH], F32, name="ns_sb")
            nc.vector.reciprocal(ns_sb[:], ns_t[:])

            with tc.tile_pool(name="ow_p", bufs=2) as ow_p, tc.tile_pool(
                name="oo_p", bufs=3
            ) as oo_p, tc.tile_pool(name="ps_o", bufs=3, space="PSUM") as ps_o:
                for n in range(4):
                    ow_h = [ow_p.tile([128, 8, 512], BF, name="ow_h")
                            for _ in range(2)]
                    for h in range(2):
                        nc.sync.dma_start(
                            ow_h[h][:],
                            owT[h * 1024 : (h + 1) * 1024,
                                n * 512 : (n + 1) * 512].rearrange(
                                "(ko p) f -> p ko f", p=128
                            ),
                        )
                    for m in range(NCH):
                        po = ps_o.tile([128, 512], F32, name="po")
                        for k in range(16):
                            nc.tensor.matmul(
                                po[:],
                                ys_sb[:, k, m * 128 : (m + 1) * 128],
                                ow_h[k // 8][:, k % 8, :],
                                start=(k == 0),
                                stop=(k == 15),
                            )
                        oo_t = oo_p.tile([128, 512], F32, name="oo_t")
                        nc.scalar.mul(oo_t[:], po[:], ns_sb[:, m : m + 1])
                        nc.sync.dma_start(
                            out[m * 128 : (m + 1) * 128, n * 512 : (n + 1) * 512],
                            oo_t[:],
                        )
        _ys_ctx.close()
        _aq_ctx.close()
        _qk_ctx.close()
        _pf_ctx.close()
        _v_ctx.close()
        free_bdm()
        free_swm()
        free_kdm()
        free_eps()
        free_ones_b()
        free_ones_f()
        free_ident()
    nc.finalize()
    return nc


_CACHE = {}


def _get_nc():
    if "nc" not in _CACHE:
        _CACHE["nc"] = _build_nc()
    return _CACHE["nc"]


def _host_prep(hidden_states, qkv_w, out_w, gate_w, norm_w):
    slope, qd, kd, dd, bd = _decay()
    w3 = qkv_w.reshape(H, 3 * D, HID)
    wq = w3[:, 0:D, :].reshape(H * D, HID)
    wk = w3[:, D : 2 * D, :].reshape(H * D, HID)
    wv = w3[:, 2 * D : 3 * D, :].reshape(H * D, HID)
    wqT = np.ascontiguousarray(wq.T.astype(BF_NP))
    wkT = np.ascontiguousarray(wk.T.astype(BF_NP))
    wvT = np.ascontiguousarray(wv.T.astype(BF_NP))
    gwT = np.ascontiguousarray(gate_w.T.astype(BF_NP))
    # norm_w folded into the output projection (rmsnorm's per-token scale
    # is applied at PSUM evacuation; the per-feature nw scales ow columns)
    owT = np.ascontiguousarray((out_w * norm_w[None, :]).T.astype(BF_NP))
    # ddm[c, p, e, jc, i] = dd[2c+e, i, jc*128+p]
    ddm = np.ascontiguousarray(
        dd.reshape(NP, 2, BS, 2, 128).transpose(0, 4, 1, 3, 2).astype(np.float32)
    )
    # qdm[p, c, i] = qd[2c + (p>=64), i]
    qdm_pair = qd.reshape(NP, 2, BS)                       # (c, e, i)
    qdm = np.ascontiguousarray(
        np.broadcast_to(
            qdm_pair.transpose(1, 0, 2)[:, None, :, :], (2, 64, NP, BS)
        ).reshape(128, NP, BS).astype(BF_NP)
    )
    kdm = np.ascontiguousarray(
        kd.reshape(H, 2, 128).transpose(2, 0, 1).reshape(128, 2 * H).astype(np.float32)
    )
    # bdm[p, c, jb] = bd[2c + (p>=64)]^jb
    jbp = np.arange(NBLK, dtype=np.float64)
    bdp = bd[:, None] ** jbp[None, :]                      # (H, NBLK)
    bdm = np.ascontiguousarray(
        np.broadcast_to(
            bdp.reshape(NP, 2, NBLK).transpose(1, 0, 2)[:, None, :, :],
            (2, 64, NP, NBLK),
        ).reshape(128, NP, NBLK).astype(np.float32)
    )

    shared = dict(wqT=wqT, wkT=wkT, wvT=wvT, gwT=gwT, owT=owT, ddm=ddm,
                  qdm=qdm, kdm=kdm, bdm=bdm)
    in_maps = []
    for c in range(NC):
        bb, p = c // 4, c % 4
        hsT = np.ascontiguousarray(
            hidden_states[bb, p * T : (p + 1) * T, :].T.astype(BF_NP)
        )
        # swm[p_, c_*8+cc] = weight of core cc for head 2c_+(p_>=64)
        sw = np.zeros((H, NC), dtype=np.float64)
        for cc in range(NC):
            if cc // 4 == bb and cc % 4 < p:
                sw[:, cc] = bd ** (4.0 * (p - 1 - (cc % 4)))
        swm = np.ascontiguousarray(
            np.broadcast_to(
                sw.reshape(NP, 2, NC).transpose(1, 0, 2)[:, None, :, :],
                (2, 64, NP, NC),
            ).reshape(128, NP * 8).astype(np.float32)
        )
        in_maps.append(dict(hsT=hsT, swm=swm, **shared))
    return in_maps


def _run(inputs, trace=False):
    nc = _get_nc()
    in_maps = _host_prep(
        np.asarray(inputs["hidden_states"], dtype=np.float32),
        np.asarray(inputs["qkv_w"], dtype=np.float32),
        np.asarray(inputs["out_w"], dtype=np.float32),
        np.asarray(inputs["gate_w"], dtype=np.float32),
        np.asarray(inputs["norm_w"], dtype=np.float32),
    )
    res = run_bass_kernel_spmd(nc, in_maps, core_ids=list(range(NC)), trace=trace)
    full = np.empty((B, S, HID), dtype=np.float32)
    for c in range(NC):
        bb, p = c // 4, c % 4
        full[bb, p * T : (p + 1) * T, :] = res.results[c]["out"]
    return full, res


def kernel(**inputs):
    return _run(inputs, trace=False)[0]


def kernel_traced(**inputs):
    full, res = _run(inputs, trace=True)
    return full, res.exec_time_ns
